# revision 12
# baseline (speedup 1.0000x reference)
"""Trainium2 Bass kernel for the sparse-attention decoder problem.

Math (per batch b):
  fixed_context = mean_n(emb) @ W_context                       [H]
  K|V|LK        = emb @ W_kvlogit (split in 3)                  [N,H] each
  query         = fixed_context + [gather(emb,cur)|feat3] @ W_step
  per head h:   compat = (Q_h K_h^T)/8 ; softmax over masked N
  heads_out     = attn @ V_h ; glimpse = heads @ W_out
  logits        = tanh(glimpse LK^T / sqrt(H)) * 10 ; mask ; log_softmax

v3 design (vs v2 at ~482us): same math/dataflow as v2 (fp8-DR projections,
transposed [n,t] attention, PE-seeded additive mask, ones-column denominator,
u@embT logits), but the elementwise/evacuation side is restructured around
MEASURED per-instruction overheads (DVE ~195ns + ACT ~242ns fixed per op):
  - PSUM is reorganized into one 6-bank "big" rotation (2-bank slots,
    bufs=3) carrying kt/v/q/cm/g/u/lg tiles, so m-chunk PAIRS evacuate as
    single [128,1024] instructions (half the instruction count of v2) and
    the PE never ping-pongs on a 2-deep rotation.
  - compat PSUM is a 2-bank tile per HEAD PAIR; one exp instruction
    [128,1024] evacuates both heads (4 exps/batch instead of 8).
  - fixed_context enters the query as a rank-2 PE seed matmul
    (fct2[2,128] x sel[2,2T]) instead of 8 per-(m,j) ACT bias ops; the
    query evacuates as 2 plain scale-copies per pair.
  - A@V normalization: reciprocal straight off the PSUM ones-column, then
    ONE broadcast (stride-0) tensor_tensor per 4-head group (HW-verified).
  - the logits tail is pair-merged: one tanh / one mask-add
    (scalar_tensor_tensor, exact -1e8 f32) / one final output op per PAIR
    of batches on [128,2,N] tiles; the Mitchell+Newton ln runs width-2.
  - transposes copy out via 2x[128,2,T] ACT ops instead of 4x[128,T].

HW facts this build is tuned against (measured via micro.py on the actual
device, not the cost model):
  - fp8 matmuls stream ~1 col/cycle regardless of DR (203ns / 512 cols);
    DoublePixel/DoubleColumn give no real speedup.
  - DVE [128,512] PSUM->SBUF copy = 629ns; ACT = 738ns. Per-op fixed
    overhead dominates small ops -> merge everything mergeable.
  - GPSIMD/Pool: 3.6us per [128,1024] op, cannot read PSUM. Useless here.
  - engine-written PSUM + matmul start=False accumulates exactly (enables
    future PE-seed offload), broadcast APs work on DVE.

Sharding: pure data-parallel over batch, 32 batches per core on 8 cores.
"""

import os
import numpy as np
import ml_dtypes
from contextlib import ExitStack

# the axon client in this image has no NTFF hook; a stray BASS_TRACE=1
# would crash run_bass_kernel_spmd, so pin tracing off for the exec path.
os.environ.setdefault("BASS_NEVER_TRACE", "1")

import concourse.bass as bass
import concourse.tile as tile
from concourse import bacc, masks, mybir
from concourse.bass_utils import run_bass_kernel_spmd

B, N, D, H, HEADS, KEY, T = 256, 512, 512, 512, 8, 64, 128
NCORES = 8
BL = B // NCORES          # batches per core
DC = D // 128             # 4 d-chunks
KQ = 6                    # padded D+3 -> 768 rows for the step projection
MA = -1e8                 # additive mask (underflows exp to 0)
MSEED = -30.0             # mask units for the compat seed (exp(-30+|c|)~0)
F32 = mybir.dt.float32
BF16 = mybir.dt.bfloat16
F8 = mybir.dt.float8e4
I32 = mybir.dt.int32
OP = mybir.AluOpType
AF = mybir.ActivationFunctionType
DR = mybir.MatmulPerfMode.DoubleRow

LAST_EXEC_TIME_NS = None

# emission-order knobs (sim-swept; see simsweep.py)
ATT_INTERLEAVE = False   # weave the two batches' head-pair chains


def _emit(ctx, tc, io, bl, loop_reps=1):
    nc = tc.nc
    (emb8, nn8, mT8, mab2, fct2d, wkv8, wstep8, wout8, wlkT8, seedw,
     selw, outp) = io

    wp = ctx.enter_context(tc.tile_pool(name="wp", bufs=1))
    wkv_t = wp.tile([128, DC, 3 * H], F8, name="wkv")
    nc.sync.dma_start(wkv_t[:], wkv8)
    wstep_t = wp.tile([128, KQ, H], F8, name="wstep")
    nc.sync.dma_start(wstep_t[:], wstep8)
    wout_t = wp.tile([128, DC, H], F8, name="wout")
    nc.sync.dma_start(wout_t[:], wout8)
    wlk_t = wp.tile([128, DC, H], F8, name="wlk")
    nc.sync.dma_start(wlk_t[:], wlkT8)
    seed_t = wp.tile([128, 2, 128], F8, name="seed")
    nc.sync.dma_start(seed_t[:], seedw)
    ident = wp.tile([128, 128], BF16, name="ident")
    masks.make_identity(nc, ident[:])
    # sel[j, jT:(j+1)T] = 1 else 0 : rank-2 fc seed selector (host const)
    sel_t = wp.tile([2, 2 * T], BF16, name="sel")
    nc.sync.dma_start(sel_t[:], selw)

    sb = ctx.enter_context(tc.tile_pool(name="sb", bufs=1))
    psb = ctx.enter_context(tc.tile_pool(name="psb", bufs=3, space="PSUM"))
    psav = ctx.enter_context(tc.tile_pool(name="psav", bufs=2, space="PSUM"))

    LN2 = float(np.log(2.0))

    def stage_abc(p, tail_late=()):
        """Projections, query, attention for pair p. Returns tail state."""
        bs = (2 * p, 2 * p + 1)
        et8, kt_sb, v2_sb, mT_sb = {}, {}, {}, {}
        for b in bs:
            et = sb.tile([128, DC, N], F8, tag="et", bufs=10, name=f"et{b}")
            nc.sync.dma_start(et[:], emb8[b])
            et8[b] = et
            mT_t = sb.tile([128, 2, 4 * T], F8, tag="mT", bufs=4, name=f"mT{b}")
            nc.sync.dma_start(mT_t[:], mT8[b])
            mT_sb[b] = mT_t
        mab_t = sb.tile([128, 2, N], BF16, tag="mab", bufs=4, name=f"mab{p}")
        nc.sync.dma_start(mab_t[:], mab2[p])
        nnq = sb.tile([128, KQ, 2 * T], F8, tag="nnq", bufs=3, name=f"nnq{p}")
        for j, b in enumerate(bs):
            nc.sync.dma_start(nnq[:, :, j * T : (j + 1) * T], nn8[b])
        fct = sb.tile([2, DC, 128], BF16, tag="fct", bufs=3, name=f"fct{p}")
        nc.sync.dma_start(fct[:], fct2d[p])

        # ---- K^T and V projections, evacuated as m-chunk pairs ----
        lateq = list(tail_late)
        for b in bs:
            kt = sb.tile([128, 2, 2, N], BF16, tag="kt", bufs=4, name=f"kt{b}")
            for mp in range(2):
                kt_ps = psb.tile([128, 2, N], F32, tag="big",
                                 name=f"ktps{b}{mp}")
                for mi in range(2):
                    m = 2 * mp + mi
                    for c in range(2):
                        nc.tensor.matmul(
                            kt_ps[:, mi, :],
                            wkv_t[:, 2 * c : 2 * c + 2, bass.ts(m, 128)],
                            et8[b][:, 2 * c : 2 * c + 2, :],
                            start=(c == 0), stop=(c == 1), perf_mode=DR)
                nc.vector.tensor_copy(kt[:, mp], kt_ps[:])
                if lateq:
                    lateq.pop(0)()
            kt_sb[b] = kt
            v2 = sb.tile([128, DC, HEADS, KEY + 1], F8, tag="v2", bufs=4,
                         name=f"v2{b}")
            nc.vector.memset(v2[:, :, :, KEY : KEY + 1], 1.0)
            for mp in range(2):
                v_ps = psb.tile([128, 2, HEADS, KEY], F32, tag="big",
                                name=f"vps{b}{mp}")
                for mi in range(2):
                    m = 2 * mp + mi
                    for c in range(2):
                        nc.tensor.matmul(
                            v_ps[:, mi],
                            et8[b][:, 2 * c : 2 * c + 2, bass.ts(m, 128)],
                            wkv_t[:, 2 * c : 2 * c + 2, H : 2 * H],
                            start=(c == 0), stop=(c == 1), perf_mode=DR)
                nc.vector.tensor_copy(v2[:, 2 * mp : 2 * mp + 2, :, 0:KEY],
                                      v_ps[:])
                if lateq:
                    lateq.pop(0)()
            v2_sb[b] = v2

        # ---- query: fc enters as a rank-2 seed matmul; plain scale out ----
        for ch in lateq:
            ch()
        qt_sb = []
        for mp in range(2):
            q_ps = psb.tile([128, 2, 2 * T], F32, tag="big",
                            name=f"qps{p}{mp}")
            for mi in range(2):
                m = 2 * mp + mi
                nc.tensor.matmul(q_ps[:, mi], fct[:, m, :], sel_t[:],
                                 start=True, stop=False,
                                 skip_group_check=True)
                for c in range(3):
                    nc.tensor.matmul(
                        q_ps[:, mi],
                        wstep_t[:, 2 * c : 2 * c + 2, bass.ts(m, 128)],
                        nnq[:, 2 * c : 2 * c + 2, :],
                        start=False, stop=(c == 2), perf_mode=DR,
                        skip_group_check=True)
            qt = sb.tile([128, 2, 2 * T], BF16, tag="qt", bufs=4,
                         name=f"qt{p}{mp}")
            nc.scalar.activation(qt[:], q_ps[:], AF.Identity, scale=0.125)
            qt_sb.append(qt)

        return (bs, et8, mT_sb, mab_t, kt_sb, v2_sb, qt_sb)

    def stage_att(p, front, tail):
        # ---- attention, both batches INTERLEAVED at the head-pair level so
        # the seed/QK (PE) of one batch overlaps the exp (ACT) and A@V of
        # the other; psav is a 2-deep rotation shared with the transposes.
        bs, et8, mT_sb, mab_t, kt_sb, v2_sb, qt_sb = front
        hd8 = sb.tile([128, DC, 2 * T], F8, tag="hd8", bufs=4, name=f"hd8{p}")
        hdn_sb, hd2_sb = {}, {}
        for b in bs:
            hdn_sb[b] = sb.tile([128, HEADS, KEY], BF16, tag="hdn", bufs=3,
                                name=f"hdn{b}")
        if ATT_INTERLEAVE:
            order = [(hp, j) for hp in range(HEADS // 2)
                     for j in range(2)]
        else:
            order = [(hp, j) for j in range(2)
                     for hp in range(HEADS // 2)]
        unit = 0
        for hp, j in order:
            b = bs[j]
            if True:
                if unit > 0 and unit - 1 < len(tail):
                    tail[unit - 1]()
                unit += 1
                cm = psb.tile([128, 2, DC, T], F32, tag="big",
                              name=f"cm{b}{hp}")
                for hl in range(2):
                    o = hl * 64
                    nc.tensor.matmul(cm[:, hl], seed_t[o : o + 64, :, :],
                                     mT_sb[b][o : o + 64, :, :],
                                     start=True, stop=False, perf_mode=DR,
                                     skip_group_check=True)
                for cn in range(DC):
                    for hl in range(2):
                        o = hl * 64
                        nc.tensor.matmul(
                            cm[:, hl, cn, :],
                            kt_sb[b][o : o + 64, hp // 2, hp % 2,
                                     bass.ts(cn, 128)],
                            qt_sb[hp // 2][o : o + 64, hp % 2,
                                           j * T : (j + 1) * T],
                            start=False, stop=(cn == DC - 1),
                            skip_group_check=True)
                pt = sb.tile([128, 2, DC, T], F8, tag="pt", bufs=6,
                             name=f"pt{b}{hp}")
                nc.scalar.activation(pt[:], cm[:], AF.Exp)
                if hp % 2 == 0:
                    hd2_sb[b] = psav.tile([128, 4, KEY + 1], F32, tag="av",
                                          padded_shape=[128, 4, 128],
                                          name=f"hd2{b}{hp // 2}")
                hd2 = hd2_sb[b]
                for hl in range(2):
                    h = 2 * hp + hl
                    hq = h % 4
                    for c in range(2):
                        nc.tensor.matmul(
                            hd2[:, hq, :],
                            pt[:, hl, 2 * c : 2 * c + 2, :],
                            v2_sb[b][:, 2 * c : 2 * c + 2, h, :],
                            start=(c == 0), stop=(c == 1), perf_mode=DR)
                if hp % 2 == 1:
                    g = hp // 2
                    rs = sb.tile([128, 4, 1], F32, tag="rs", bufs=4,
                                 name=f"rs{b}{g}")
                    nc.vector.reciprocal(rs[:], hd2[:, :, KEY : KEY + 1])
                    _, rb = bass.broadcast_tensor_aps(hd2[:, :, 0:KEY], rs[:])
                    nc.vector.tensor_tensor(
                        hdn_sb[b][:, 4 * g : 4 * g + 4, :],
                        hd2[:, :, 0:KEY], rb, op=OP.mult)
        for k in range(unit - 1, len(tail)):
            tail[k]()
        # transpose heads_out [t,hk] -> [hk,t]; emitted after all head chains
        for j, b in enumerate(bs):
            for half in range(2):
                tp = psav.tile([128, 2, T], BF16, tag="av", name=f"tp{b}{half}")
                for i in range(2):
                    c = 2 * half + i
                    nc.tensor.transpose(tp[:, i, :],
                                        hdn_sb[b][:, 2 * c : 2 * c + 2, :],
                                        ident[:])
                nc.scalar.copy(
                    hd8[:, 2 * half : 2 * half + 2, j * T : (j + 1) * T],
                    tp[:])
        return (p, bs, et8, mab_t, hd8)

    def stage_d_chunks(state):
        """Glimpse, u, logits, log_softmax for a previously emitted pair —
        returned as a list of emission closures so stage_att can weave them
        between head-group units (fills each engine's dependency stalls with
        ready tail work)."""
        p, bs, et8, mab_t, hd8 = state
        st = {}
        chunks = []

        def c_g(mp):
            g8 = st.setdefault("g8", sb.tile([128, DC, 2 * T], F8, tag="g8",
                                             bufs=3, name=f"g8{p}"))
            g_ps = psb.tile([128, 2, 2 * T], F32, tag="big",
                            name=f"gps{p}{mp}")
            for mi in range(2):
                m = 2 * mp + mi
                for c in range(2):
                    nc.tensor.matmul(
                        g_ps[:, mi],
                        wout_t[:, 2 * c : 2 * c + 2, bass.ts(m, 128)],
                        hd8[:, 2 * c : 2 * c + 2, :],
                        start=(c == 0), stop=(c == 1), perf_mode=DR)
            nc.vector.tensor_copy(g8[:, 2 * mp : 2 * mp + 2], g_ps[:])

        def c_u(mp):
            u8 = st.setdefault("u8", sb.tile([128, DC, 2 * T], BF16, tag="u8",
                                             bufs=3, name=f"u8{p}"))
            u_ps = psb.tile([128, 2, 2 * T], F32, tag="big",
                            name=f"ups{p}{mp}")
            for mi in range(2):
                m = 2 * mp + mi
                for c in range(2):
                    nc.tensor.matmul(
                        u_ps[:, mi],
                        wlk_t[:, 2 * c : 2 * c + 2, bass.ts(m, 128)],
                        st["g8"][:, 2 * c : 2 * c + 2, :],
                        start=(c == 0), stop=(c == 1), perf_mode=DR)
            nc.vector.tensor_copy(u8[:, 2 * mp : 2 * mp + 2], u_ps[:])

        def c_lg():
            lg = psb.tile([128, 2, N], F32, tag="big", name=f"lg{p}")
            for j, b in enumerate(bs):
                for c in range(DC):
                    nc.tensor.matmul(lg[:, j],
                                     st["u8"][:, c, j * T : (j + 1) * T],
                                     et8[b][:, c, :],
                                     start=(c == 0), stop=(c == DC - 1),
                                     skip_group_check=True)
            st["lg"] = lg

        def c_y():
            y2 = sb.tile([128, 2, N], F32, tag="y2", bufs=2, name=f"y2{p}")
            nc.scalar.activation(y2[:], st["lg"][:], AF.Tanh,
                                 scale=float(1.0 / np.sqrt(H)))
            # t2 = y + mask * (-1e8), exact f32 mask units
            t2 = sb.tile([128, 2, N], F32, tag="t2", bufs=2, name=f"t2{p}")
            nc.vector.scalar_tensor_tensor(t2[:], mab_t[:], float(MA), y2[:],
                                           op0=OP.mult, op1=OP.add)
            st["t2"] = t2

        def c_p2():
            t2 = st["t2"]
            p2 = sb.tile([128, N], BF16, tag="p2", bufs=2, name=f"p2{p}")
            s2 = sb.tile([128, 2], F32, tag="s2", bufs=2, name=f"s2{p}")
            for j in range(2):
                nc.scalar.activation(p2[:], t2[:, j], AF.Exp, scale=10.0,
                                     accum_out=s2[:, j : j + 1])
            st["s2"] = s2

        def c_out():
            s2, t2 = st["s2"], st["t2"]
            # ln(s2) width-2: Mitchell bit-trick seed + 2 Newton steps
            lns = sb.tile([128, 2, 1], F32, tag="lns", bufs=4, name=f"lns{p}")
            nc.vector.tensor_scalar(lns[:, :, 0], s2[:].bitcast(I32),
                                    LN2 / (1 << 23), (127.0 - 0.0430) * LN2,
                                    op0=OP.mult, op1=OP.subtract)
            for it in range(2):
                ex = sb.tile([128, 2], F32, tag="nex", bufs=4,
                             name=f"nex{p}{it}")
                nc.scalar.activation(ex[:], lns[:, :, 0], AF.Exp, scale=-1.0)
                tmp = sb.tile([128, 2], F32, tag="ntmp", bufs=4,
                              name=f"ntmp{p}{it}")
                nc.vector.scalar_tensor_tensor(tmp[:], ex[:], 1.0, s2[:],
                                               op0=OP.mult, op1=OP.mult)
                ln2t = sb.tile([128, 2, 1], F32, tag="lns", bufs=4,
                               name=f"lns{p}_{it}")
                nc.vector.scalar_tensor_tensor(ln2t[:, :, 0], tmp[:], -1.0,
                                               lns[:, :, 0],
                                               op0=OP.add, op1=OP.add)
                lns = ln2t
            # o = 10*t2 - lns  (lns broadcast along N)
            o2 = sb.tile([128, 2, N], F32, tag="o2", bufs=2, name=f"o2{p}")
            _, lb = bass.broadcast_tensor_aps(t2[:], lns[:])
            nc.vector.scalar_tensor_tensor(o2[:], t2[:], 10.0, lb,
                                           op0=OP.mult, op1=OP.subtract)
            for j, b in enumerate(bs):
                nc.sync.dma_start(outp[b], o2[:, j])

        chunks = [lambda: c_g(0), lambda: c_g(1), lambda: c_u(0),
                  lambda: c_u(1), c_lg, c_y, c_p2, c_out]
        return chunks

    # software pipeline: emit pair p's tail after pair p+1's front half so
    # the in-order per-engine queues never head-of-line block on the serial
    # logits/log_softmax chain.
    def pair_loop():
        P = bl // 2
        chunks = {}
        for p in range(P):
            front = stage_abc(p, chunks[p - 2][4:8] if p >= 2 else ())
            early = chunks[p - 1][0:4] if p >= 1 else []
            state = stage_att(p, front, early)
            chunks[p] = stage_d_chunks(state)
        for ch in chunks[P - 2][4:8]:
            ch()
        for ch in chunks[P - 1]:
            ch()

    if loop_reps > 1:
        with tc.For_i(0, loop_reps):
            pair_loop()
    else:
        pair_loop()


def _build(bl, reps=1, hwloop=False):
    nc = bacc.Bacc("TRN2", target_bir_lowering=False, debug=False)
    emb8 = nc.dram_tensor("emb8", [bl, 128, DC, N], F8, kind="ExternalInput").ap()
    nn8 = nc.dram_tensor("nn8", [bl, 128, KQ, T], F8, kind="ExternalInput").ap()
    mT8 = nc.dram_tensor("mT8", [bl, 128, 2, 4 * T], F8, kind="ExternalInput").ap()
    mab2 = nc.dram_tensor("mab2", [bl // 2, 128, 2, N], BF16,
                          kind="ExternalInput").ap()
    fct2d = nc.dram_tensor("fct2d", [bl // 2, 2, DC, 128], BF16,
                           kind="ExternalInput").ap()
    wkv8 = nc.dram_tensor("wkv8", [128, DC, 3 * H], F8, kind="ExternalInput").ap()
    wstep8 = nc.dram_tensor("wstep8", [128, KQ, H], F8, kind="ExternalInput").ap()
    wout8 = nc.dram_tensor("wout8", [128, DC, H], F8, kind="ExternalInput").ap()
    wlkT8 = nc.dram_tensor("wlkT8", [128, DC, H], F8, kind="ExternalInput").ap()
    seedw = nc.dram_tensor("seedw", [128, 2, 128], F8, kind="ExternalInput").ap()
    selw = nc.dram_tensor("selw", [2, 2 * T], BF16, kind="ExternalInput").ap()
    outp = nc.dram_tensor("logp", [bl, T, N], F32, kind="ExternalOutput").ap()
    with tile.TileContext(nc) as tc:
        if hwloop:
            with ExitStack() as ctx:
                _emit(ctx, tc, (emb8, nn8, mT8, mab2, fct2d, wkv8, wstep8,
                                wout8, wlkT8, seedw, selw, outp), bl,
                      loop_reps=reps)
        else:
            for _ in range(reps):
                with ExitStack() as ctx:
                    _emit(ctx, tc, (emb8, nn8, mT8, mab2, fct2d, wkv8,
                                    wstep8, wout8, wlkT8, seedw, selw,
                                    outp), bl)
    nc.compile()
    return nc


_cache = {}


def _program(bl, reps=1, hwloop=False):
    key = (bl, reps, hwloop)
    if key not in _cache:
        _cache[key] = _build(bl, reps, hwloop)
    return _cache[key]


def _f8(a):
    return a.astype(mybir.dt.np(F8))


def _prep(embedding, current_nodes, used_capacity, used_battery, current_time,
          mask, W_context):
    b = embedding.shape[0]
    # emb8[b,p,c,n] = emb[b, n, c*128+p]
    embT = np.ascontiguousarray(embedding.transpose(0, 2, 1))  # [B, D, N]
    emb8 = _f8(embT.reshape(b, DC, 128, N).transpose(0, 2, 1, 3))
    # nn8[b,p,c,t] = feat[b, t, c*128+p], rows >= D+3 zero
    cur = np.take_along_axis(embedding, current_nodes.astype(np.int64)[:, :, None],
                             axis=1)
    nnf = np.zeros((b, KQ * 128, T), np.float32)
    nnf[:, :D, :] = cur.transpose(0, 2, 1)
    nnf[:, D, :] = 1.0 - used_capacity
    nnf[:, D + 1, :] = 1.0 - used_battery
    nnf[:, D + 2, :] = current_time
    nn8 = _f8(nnf.reshape(b, KQ, 128, T).transpose(0, 2, 1, 3))
    # mT8[b, k or 64+k, i, c*T+t] = MSEED * mask[b, t, c*128 + k + 64*i]
    maT = mask.transpose(0, 2, 1).astype(np.float32) * np.float32(MSEED)
    mT = maT.reshape(b, DC, 2, 64, T).transpose(0, 3, 2, 1, 4).reshape(b, 64, 2, 4 * T)
    mT8 = _f8(np.concatenate([mT, mT], axis=1))  # duplicate rows for PE pairing
    # mab2[pair, t, j, n] = mask[2*pair+j, t, n]  (0/1; scaled -1e8 on device)
    mab2 = np.ascontiguousarray(
        mask.reshape(b // 2, 2, T, N).transpose(0, 2, 1, 3)
    ).astype(ml_dtypes.bfloat16)
    # host fixed context (unscaled; device applies the 1/8 with the query):
    # fct2d[pair, j, m, i] = fc[2*pair + j, m*128 + i]
    fc = (embedding.mean(axis=1) @ W_context).astype(np.float32)  # [B, H]
    fct2d = fc.reshape(b // 2, 2, DC, 128).astype(ml_dtypes.bfloat16)
    return emb8, nn8, mT8, mab2, fct2d


def _prep_weights(W_kvlogit, W_step, W_out):
    wkv8 = _f8(W_kvlogit.reshape(DC, 128, 3 * H).transpose(1, 0, 2))
    ws = np.zeros((KQ * 128, H), np.float32)
    ws[: D + 3] = W_step
    wstep8 = _f8(ws.reshape(KQ, 128, H).transpose(1, 0, 2))
    wout8 = _f8(W_out.reshape(DC, 128, H).transpose(1, 0, 2))
    # wlkT8[p,c,d] = W_lk[d, c*128+p]
    wlk = W_kvlogit[:, 2 * H :]  # [D, H]
    wlkT8 = _f8(np.ascontiguousarray(wlk.T).reshape(DC, 128, D).transpose(1, 0, 2))
    z = np.zeros((64, 2, 128), np.float32)
    for i in range(2):
        z[np.arange(64), i, np.arange(64) + 64 * i] = 1.0
    seedw = _f8(np.concatenate([z, z], axis=0))
    selw = np.zeros((2, 2 * T), np.float32)
    selw[0, 0:T] = 1.0
    selw[1, T:] = 1.0
    selw = selw.astype(ml_dtypes.bfloat16)
    return wkv8, wstep8, wout8, wlkT8, seedw, selw


def prep_in_maps(inputs):
    """Full harness inputs -> per-core input maps for the device program."""
    embedding = np.asarray(inputs["embedding"], np.float32)
    mask = np.asarray(inputs["mask"], bool)
    emb8, nn8, mT8, mab2, fct2d = _prep(
        embedding, np.asarray(inputs["current_nodes"]),
        np.asarray(inputs["used_capacity"], np.float32),
        np.asarray(inputs["used_battery"], np.float32),
        np.asarray(inputs["current_time"], np.float32), mask,
        np.asarray(inputs["W_context"], np.float32))
    wkv8, wstep8, wout8, wlkT8, seedw, selw = _prep_weights(
        np.asarray(inputs["W_kvlogit"], np.float32),
        np.asarray(inputs["W_step"], np.float32),
        np.asarray(inputs["W_out"], np.float32))
    in_maps = []
    for c in range(NCORES):
        s = slice(c * BL, (c + 1) * BL)
        sp = slice(c * BL // 2, (c + 1) * BL // 2)
        in_maps.append({"emb8": emb8[s], "nn8": nn8[s], "mT8": mT8[s],
                        "mab2": mab2[sp], "fct2d": fct2d[sp],
                        "wkv8": wkv8, "wstep8": wstep8, "wout8": wout8,
                        "wlkT8": wlkT8, "seedw": seedw, "selw": selw})
    return in_maps


def kernel(embedding, current_nodes, used_capacity, used_battery, current_time,
           mask, W_context, W_kvlogit, W_step, W_out):
    global LAST_EXEC_TIME_NS
    in_maps = prep_in_maps(dict(
        embedding=embedding, current_nodes=current_nodes,
        used_capacity=used_capacity, used_battery=used_battery,
        current_time=current_time, mask=mask, W_context=W_context,
        W_kvlogit=W_kvlogit, W_step=W_step, W_out=W_out))
    nc = _program(BL)
    res = run_bass_kernel_spmd(nc, in_maps, list(range(NCORES)))
    LAST_EXEC_TIME_NS = res.exec_time_ns
    return np.concatenate([res.results[c]["logp"] for c in range(NCORES)], axis=0)


# revision 14
# speedup vs baseline: 1.1791x; 1.1791x over previous
"""Trainium2 Bass kernel for the sparse-attention decoder problem.

Math (per batch b):
  fixed_context = mean_n(emb) @ W_context                       [H]
  K|V|LK        = emb @ W_kvlogit (split in 3)                  [N,H] each
  query         = fixed_context + [gather(emb,cur)|feat3] @ W_step
  per head h:   compat = (Q_h K_h^T)/8 ; softmax over masked N
  heads_out     = attn @ V_h ; glimpse = heads @ W_out
  logits        = tanh(glimpse LK^T / sqrt(H)) * 10 ; mask ; log_softmax

v3 design (vs v2 at ~482us; this build ~433us): same math/dataflow as v2
(fp8-DR projections, transposed [n,t] attention, PE-seeded additive mask,
ones-column denominator, u@embT logits), but the elementwise/evacuation side
is restructured around MEASURED per-instruction overheads (DVE ~195ns +
ACT ~242ns fixed per op), and the tail is software-pipelined INTO the next
pair's attention emission as 8 chunks (see stage_d_chunks/stage_att): each
chunk lands between two head-pair units so every engine's exp/AV dependency
stall is filled with ready tail work. Tried and reverted (HW-measured):
splitting the tail weave across the projection window too (497us - the lg
allocations in the big rotation stall the next kt groups); block-diagonal
merged seed matmuls (matmul output cannot cross a PSUM bank boundary);
folding the mask into the QK contraction as extra DR rows (the stationary
operand would need the mask replicated 16x -> DMA-bound). Elementwise
changes:
  - PSUM is reorganized into one 6-bank "big" rotation (2-bank slots,
    bufs=3) carrying kt/v/q/cm/g/u/lg tiles, so m-chunk PAIRS evacuate as
    single [128,1024] instructions (half the instruction count of v2) and
    the PE never ping-pongs on a 2-deep rotation.
  - compat PSUM is a 2-bank tile per HEAD PAIR; one exp instruction
    [128,1024] evacuates both heads (4 exps/batch instead of 8).
  - fixed_context enters the query as a rank-2 PE seed matmul
    (fct2[2,128] x sel[2,2T]) instead of 8 per-(m,j) ACT bias ops; the
    query evacuates as 2 plain scale-copies per pair.
  - A@V normalization: reciprocal straight off the PSUM ones-column, then
    ONE broadcast (stride-0) tensor_tensor per 4-head group (HW-verified).
  - the logits tail is pair-merged: one tanh / one mask-add
    (scalar_tensor_tensor, exact -1e8 f32) / one final output op per PAIR
    of batches on [128,2,N] tiles; the Mitchell+Newton ln runs width-2.
  - transposes copy out via 2x[128,2,T] ACT ops instead of 4x[128,T].

HW facts this build is tuned against (measured via micro.py on the actual
device, not the cost model):
  - fp8 matmuls stream ~1 col/cycle regardless of DR (203ns / 512 cols);
    DoublePixel/DoubleColumn give no real speedup.
  - DVE [128,512] PSUM->SBUF copy = 629ns; ACT = 738ns. Per-op fixed
    overhead dominates small ops -> merge everything mergeable.
  - GPSIMD/Pool: 3.6us per [128,1024] op, cannot read PSUM. Useless here.
  - engine-written PSUM + matmul start=False accumulates exactly (enables
    future PE-seed offload), broadcast APs work on DVE.

Sharding: pure data-parallel over batch, 32 batches per core on 8 cores.
"""

import os
import numpy as np
import ml_dtypes
from contextlib import ExitStack

# the axon client in this image has no NTFF hook; a stray BASS_TRACE=1
# would crash run_bass_kernel_spmd, so pin tracing off for the exec path.
os.environ.setdefault("BASS_NEVER_TRACE", "1")

import concourse.bass as bass
import concourse.tile as tile
from concourse import bacc, masks, mybir
from concourse.bass_utils import run_bass_kernel_spmd

B, N, D, H, HEADS, KEY, T = 256, 512, 512, 512, 8, 64, 128
NCORES = 8
BL = B // NCORES          # batches per core
DC = D // 128             # 4 d-chunks
KQ = 6                    # padded D+3 -> 768 rows for the step projection
MA = -1e8                 # additive mask (underflows exp to 0)
MSEED = -30.0             # mask units for the compat seed (exp(-30+|c|)~0)
F32 = mybir.dt.float32
BF16 = mybir.dt.bfloat16
F8 = mybir.dt.float8e4
I32 = mybir.dt.int32
OP = mybir.AluOpType
AF = mybir.ActivationFunctionType
DR = mybir.MatmulPerfMode.DoubleRow

LAST_EXEC_TIME_NS = None

# emission-order knobs (sim-swept; see simsweep.py)
ATT_INTERLEAVE = False   # weave the two batches' head-pair chains
TAIL_DEPTH = 1           # tail of pair p-TAIL_DEPTH weaves into attention(p)


def _emit(ctx, tc, io, bl, loop_reps=1):
    nc = tc.nc
    (emb8, nn8, mT8, mab2, fct2d, wkv8, wstep8, wout8, wlkT8, seedw,
     selw, outp) = io

    wp = ctx.enter_context(tc.tile_pool(name="wp", bufs=1))
    wkv_t = wp.tile([128, DC, 3 * H], F8, name="wkv")
    nc.sync.dma_start(wkv_t[:], wkv8)
    wstep_t = wp.tile([128, KQ, H], F8, name="wstep")
    nc.sync.dma_start(wstep_t[:], wstep8)
    wout_t = wp.tile([128, DC, H], F8, name="wout")
    nc.sync.dma_start(wout_t[:], wout8)
    wlk_t = wp.tile([128, DC, H], F8, name="wlk")
    nc.sync.dma_start(wlk_t[:], wlkT8)
    seed_t = wp.tile([128, 2, 128], F8, name="seed")
    nc.sync.dma_start(seed_t[:], seedw)
    ident = wp.tile([128, 128], BF16, name="ident")
    masks.make_identity(nc, ident[:])
    # sel[j, jT:(j+1)T] = 1 else 0 : rank-2 fc seed selector (host const)
    sel_t = wp.tile([2, 2 * T], BF16, name="sel")
    nc.sync.dma_start(sel_t[:], selw)

    sb = ctx.enter_context(tc.tile_pool(name="sb", bufs=1))
    psb = ctx.enter_context(tc.tile_pool(name="psb", bufs=3, space="PSUM"))
    psav = ctx.enter_context(tc.tile_pool(name="psav", bufs=2, space="PSUM"))

    LN2 = float(np.log(2.0))

    def stage_abc(p):
        """Projections, query, attention for pair p. Returns tail state."""
        bs = (2 * p, 2 * p + 1)
        et8, kt_sb, v2_sb, mT_sb = {}, {}, {}, {}
        for b in bs:
            et = sb.tile([128, DC, N], F8, tag="et", bufs=10, name=f"et{b}")
            nc.sync.dma_start(et[:], emb8[b])
            et8[b] = et
            mT_t = sb.tile([128, 2, 4 * T], F8, tag="mT", bufs=4, name=f"mT{b}")
            nc.sync.dma_start(mT_t[:], mT8[b])
            mT_sb[b] = mT_t
        mab_t = sb.tile([128, 2, N], BF16, tag="mab", bufs=4, name=f"mab{p}")
        nc.sync.dma_start(mab_t[:], mab2[p])
        nnq = sb.tile([128, KQ, 2 * T], F8, tag="nnq", bufs=3, name=f"nnq{p}")
        for j, b in enumerate(bs):
            nc.sync.dma_start(nnq[:, :, j * T : (j + 1) * T], nn8[b])
        fct = sb.tile([2, DC, 128], BF16, tag="fct", bufs=3, name=f"fct{p}")
        nc.sync.dma_start(fct[:], fct2d[p])

        # ---- K^T and V projections, evacuated as m-chunk pairs ----
        for b in bs:
            kt = sb.tile([128, 2, 2, N], BF16, tag="kt", bufs=4, name=f"kt{b}")
            for mp in range(2):
                kt_ps = psb.tile([128, 2, N], F32, tag="big",
                                 name=f"ktps{b}{mp}")
                for mi in range(2):
                    m = 2 * mp + mi
                    for c in range(2):
                        nc.tensor.matmul(
                            kt_ps[:, mi, :],
                            wkv_t[:, 2 * c : 2 * c + 2, bass.ts(m, 128)],
                            et8[b][:, 2 * c : 2 * c + 2, :],
                            start=(c == 0), stop=(c == 1), perf_mode=DR)
                nc.vector.tensor_copy(kt[:, mp], kt_ps[:])
            kt_sb[b] = kt
            v2 = sb.tile([128, DC, HEADS, KEY + 1], F8, tag="v2", bufs=4,
                         name=f"v2{b}")
            nc.vector.memset(v2[:, :, :, KEY : KEY + 1], 1.0)
            for mp in range(2):
                v_ps = psb.tile([128, 2, HEADS, KEY], F32, tag="big",
                                name=f"vps{b}{mp}")
                for mi in range(2):
                    m = 2 * mp + mi
                    for c in range(2):
                        nc.tensor.matmul(
                            v_ps[:, mi],
                            et8[b][:, 2 * c : 2 * c + 2, bass.ts(m, 128)],
                            wkv_t[:, 2 * c : 2 * c + 2, H : 2 * H],
                            start=(c == 0), stop=(c == 1), perf_mode=DR)
                nc.vector.tensor_copy(v2[:, 2 * mp : 2 * mp + 2, :, 0:KEY],
                                      v_ps[:])
            v2_sb[b] = v2

        # ---- query: fc enters as a rank-2 seed matmul; plain scale out ----
        qt_sb = []
        for mp in range(2):
            q_ps = psb.tile([128, 2, 2 * T], F32, tag="big",
                            name=f"qps{p}{mp}")
            for mi in range(2):
                m = 2 * mp + mi
                nc.tensor.matmul(q_ps[:, mi], fct[:, m, :], sel_t[:],
                                 start=True, stop=False,
                                 skip_group_check=True)
                for c in range(3):
                    nc.tensor.matmul(
                        q_ps[:, mi],
                        wstep_t[:, 2 * c : 2 * c + 2, bass.ts(m, 128)],
                        nnq[:, 2 * c : 2 * c + 2, :],
                        start=False, stop=(c == 2), perf_mode=DR,
                        skip_group_check=True)
            qt = sb.tile([128, 2, 2 * T], BF16, tag="qt", bufs=4,
                         name=f"qt{p}{mp}")
            nc.scalar.activation(qt[:], q_ps[:], AF.Identity, scale=0.125)
            qt_sb.append(qt)

        return (bs, et8, mT_sb, mab_t, kt_sb, v2_sb, qt_sb)

    def stage_att(p, front, tail):
        # ---- attention, both batches INTERLEAVED at the head-pair level so
        # the seed/QK (PE) of one batch overlaps the exp (ACT) and A@V of
        # the other; psav is a 2-deep rotation shared with the transposes.
        bs, et8, mT_sb, mab_t, kt_sb, v2_sb, qt_sb = front
        hd8 = sb.tile([128, DC, 2 * T], F8, tag="hd8", bufs=4, name=f"hd8{p}")
        hdn_sb, hd2_sb = {}, {}
        for b in bs:
            hdn_sb[b] = sb.tile([128, HEADS, KEY], BF16, tag="hdn", bufs=3,
                                name=f"hdn{b}")
        if ATT_INTERLEAVE:
            order = [(hp, j) for hp in range(HEADS // 2)
                     for j in range(2)]
        else:
            order = [(hp, j) for j in range(2)
                     for hp in range(HEADS // 2)]
        unit = 0
        for hp, j in order:
            b = bs[j]
            if True:
                if unit > 0 and unit - 1 < len(tail):
                    tail[unit - 1]()
                unit += 1
                cm = psb.tile([128, 2, DC, T], F32, tag="big",
                              name=f"cm{b}{hp}")
                for hl in range(2):
                    o = hl * 64
                    nc.tensor.matmul(cm[:, hl], seed_t[o : o + 64, :, :],
                                     mT_sb[b][o : o + 64, :, :],
                                     start=True, stop=False, perf_mode=DR,
                                     skip_group_check=True)
                for cn in range(DC):
                    for hl in range(2):
                        o = hl * 64
                        nc.tensor.matmul(
                            cm[:, hl, cn, :],
                            kt_sb[b][o : o + 64, hp // 2, hp % 2,
                                     bass.ts(cn, 128)],
                            qt_sb[hp // 2][o : o + 64, hp % 2,
                                           j * T : (j + 1) * T],
                            start=False, stop=(cn == DC - 1),
                            skip_group_check=True)
                pt = sb.tile([128, 2, DC, T], F8, tag="pt", bufs=6,
                             name=f"pt{b}{hp}")
                nc.scalar.activation(pt[:], cm[:], AF.Exp)
                if hp % 2 == 0:
                    hd2_sb[b] = psav.tile([128, 4, KEY + 1], F32, tag="av",
                                          padded_shape=[128, 4, 128],
                                          name=f"hd2{b}{hp // 2}")
                hd2 = hd2_sb[b]
                for hl in range(2):
                    h = 2 * hp + hl
                    hq = h % 4
                    for c in range(2):
                        nc.tensor.matmul(
                            hd2[:, hq, :],
                            pt[:, hl, 2 * c : 2 * c + 2, :],
                            v2_sb[b][:, 2 * c : 2 * c + 2, h, :],
                            start=(c == 0), stop=(c == 1), perf_mode=DR)
                if hp % 2 == 1:
                    g = hp // 2
                    rs = sb.tile([128, 4, 1], F32, tag="rs", bufs=4,
                                 name=f"rs{b}{g}")
                    nc.vector.reciprocal(rs[:], hd2[:, :, KEY : KEY + 1])
                    _, rb = bass.broadcast_tensor_aps(hd2[:, :, 0:KEY], rs[:])
                    nc.vector.tensor_tensor(
                        hdn_sb[b][:, 4 * g : 4 * g + 4, :],
                        hd2[:, :, 0:KEY], rb, op=OP.mult)
        for k in range(unit - 1, len(tail)):
            tail[k]()
        # transpose heads_out [t,hk] -> [hk,t]; emitted after all head chains
        for j, b in enumerate(bs):
            for half in range(2):
                tp = psav.tile([128, 2, T], BF16, tag="av", name=f"tp{b}{half}")
                for i in range(2):
                    c = 2 * half + i
                    nc.tensor.transpose(tp[:, i, :],
                                        hdn_sb[b][:, 2 * c : 2 * c + 2, :],
                                        ident[:])
                nc.scalar.copy(
                    hd8[:, 2 * half : 2 * half + 2, j * T : (j + 1) * T],
                    tp[:])
        return (p, bs, et8, mab_t, hd8)

    def stage_d_chunks(state):
        """Glimpse, u, logits, log_softmax for a previously emitted pair —
        returned as a list of emission closures so stage_att can weave them
        between head-group units (fills each engine's dependency stalls with
        ready tail work)."""
        p, bs, et8, mab_t, hd8 = state
        st = {}
        chunks = []

        def c_g(mp):
            g8 = st.setdefault("g8", sb.tile([128, DC, 2 * T], F8, tag="g8",
                                             bufs=3, name=f"g8{p}"))
            g_ps = psb.tile([128, 2, 2 * T], F32, tag="big",
                            name=f"gps{p}{mp}")
            for mi in range(2):
                m = 2 * mp + mi
                for c in range(2):
                    nc.tensor.matmul(
                        g_ps[:, mi],
                        wout_t[:, 2 * c : 2 * c + 2, bass.ts(m, 128)],
                        hd8[:, 2 * c : 2 * c + 2, :],
                        start=(c == 0), stop=(c == 1), perf_mode=DR)
            nc.vector.tensor_copy(g8[:, 2 * mp : 2 * mp + 2], g_ps[:])

        def c_u(mp):
            u8 = st.setdefault("u8", sb.tile([128, DC, 2 * T], BF16, tag="u8",
                                             bufs=3, name=f"u8{p}"))
            u_ps = psb.tile([128, 2, 2 * T], F32, tag="big",
                            name=f"ups{p}{mp}")
            for mi in range(2):
                m = 2 * mp + mi
                for c in range(2):
                    nc.tensor.matmul(
                        u_ps[:, mi],
                        wlk_t[:, 2 * c : 2 * c + 2, bass.ts(m, 128)],
                        st["g8"][:, 2 * c : 2 * c + 2, :],
                        start=(c == 0), stop=(c == 1), perf_mode=DR)
            nc.vector.tensor_copy(u8[:, 2 * mp : 2 * mp + 2], u_ps[:])

        def c_lg():
            lg = psb.tile([128, 2, N], F32, tag="big", name=f"lg{p}")
            for j, b in enumerate(bs):
                for c in range(DC):
                    nc.tensor.matmul(lg[:, j],
                                     st["u8"][:, c, j * T : (j + 1) * T],
                                     et8[b][:, c, :],
                                     start=(c == 0), stop=(c == DC - 1),
                                     skip_group_check=True)
            st["lg"] = lg

        def c_y():
            y2 = sb.tile([128, 2, N], F32, tag="y2", bufs=2, name=f"y2{p}")
            nc.scalar.activation(y2[:], st["lg"][:], AF.Tanh,
                                 scale=float(1.0 / np.sqrt(H)))
            # t2 = y + mask * (-1e8), exact f32 mask units
            t2 = sb.tile([128, 2, N], F32, tag="t2", bufs=2, name=f"t2{p}")
            nc.vector.scalar_tensor_tensor(t2[:], mab_t[:], float(MA), y2[:],
                                           op0=OP.mult, op1=OP.add)
            st["t2"] = t2

        def c_p2():
            t2 = st["t2"]
            p2 = sb.tile([128, N], BF16, tag="p2", bufs=2, name=f"p2{p}")
            s2 = sb.tile([128, 2], F32, tag="s2", bufs=2, name=f"s2{p}")
            for j in range(2):
                nc.scalar.activation(p2[:], t2[:, j], AF.Exp, scale=10.0,
                                     accum_out=s2[:, j : j + 1])
            st["s2"] = s2

        def c_out():
            s2, t2 = st["s2"], st["t2"]
            # ln(s2) width-2: Mitchell bit-trick seed + 2 Newton steps
            lns = sb.tile([128, 2, 1], F32, tag="lns", bufs=4, name=f"lns{p}")
            nc.vector.tensor_scalar(lns[:, :, 0], s2[:].bitcast(I32),
                                    LN2 / (1 << 23), (127.0 - 0.0430) * LN2,
                                    op0=OP.mult, op1=OP.subtract)
            for it in range(2):
                ex = sb.tile([128, 2], F32, tag="nex", bufs=4,
                             name=f"nex{p}{it}")
                nc.scalar.activation(ex[:], lns[:, :, 0], AF.Exp, scale=-1.0)
                tmp = sb.tile([128, 2], F32, tag="ntmp", bufs=4,
                              name=f"ntmp{p}{it}")
                nc.vector.scalar_tensor_tensor(tmp[:], ex[:], 1.0, s2[:],
                                               op0=OP.mult, op1=OP.mult)
                ln2t = sb.tile([128, 2, 1], F32, tag="lns", bufs=4,
                               name=f"lns{p}_{it}")
                nc.vector.scalar_tensor_tensor(ln2t[:, :, 0], tmp[:], -1.0,
                                               lns[:, :, 0],
                                               op0=OP.add, op1=OP.add)
                lns = ln2t
            # o = 10*t2 - lns  (lns broadcast along N)
            o2 = sb.tile([128, 2, N], F32, tag="o2", bufs=2, name=f"o2{p}")
            _, lb = bass.broadcast_tensor_aps(t2[:], lns[:])
            nc.vector.scalar_tensor_tensor(o2[:], t2[:], 10.0, lb,
                                           op0=OP.mult, op1=OP.subtract)
            for j, b in enumerate(bs):
                nc.sync.dma_start(outp[b], o2[:, j])

        chunks = [lambda: c_g(0), lambda: c_g(1), lambda: c_u(0),
                  lambda: c_u(1), c_lg, c_y, c_p2, c_out]
        return chunks

    # software pipeline: emit pair p's tail after pair p+1's front half so
    # the in-order per-engine queues never head-of-line block on the serial
    # logits/log_softmax chain.
    def pair_loop():
        pendings = []
        for p in range(bl // 2):
            front = stage_abc(p)
            tail = (stage_d_chunks(pendings.pop(0))
                    if len(pendings) >= TAIL_DEPTH else [])
            pendings.append(stage_att(p, front, tail))
        for state in pendings:
            for ch in stage_d_chunks(state):
                ch()

    if loop_reps > 1:
        with tc.For_i(0, loop_reps):
            pair_loop()
    else:
        pair_loop()


def _build(bl, reps=1, hwloop=False):
    nc = bacc.Bacc("TRN2", target_bir_lowering=False, debug=False)
    emb8 = nc.dram_tensor("emb8", [bl, 128, DC, N], F8, kind="ExternalInput").ap()
    nn8 = nc.dram_tensor("nn8", [bl, 128, KQ, T], F8, kind="ExternalInput").ap()
    mT8 = nc.dram_tensor("mT8", [bl, 128, 2, 4 * T], F8, kind="ExternalInput").ap()
    mab2 = nc.dram_tensor("mab2", [bl // 2, 128, 2, N], BF16,
                          kind="ExternalInput").ap()
    fct2d = nc.dram_tensor("fct2d", [bl // 2, 2, DC, 128], BF16,
                           kind="ExternalInput").ap()
    wkv8 = nc.dram_tensor("wkv8", [128, DC, 3 * H], F8, kind="ExternalInput").ap()
    wstep8 = nc.dram_tensor("wstep8", [128, KQ, H], F8, kind="ExternalInput").ap()
    wout8 = nc.dram_tensor("wout8", [128, DC, H], F8, kind="ExternalInput").ap()
    wlkT8 = nc.dram_tensor("wlkT8", [128, DC, H], F8, kind="ExternalInput").ap()
    seedw = nc.dram_tensor("seedw", [128, 2, 128], F8, kind="ExternalInput").ap()
    selw = nc.dram_tensor("selw", [2, 2 * T], BF16, kind="ExternalInput").ap()
    outp = nc.dram_tensor("logp", [bl, T, N], F32, kind="ExternalOutput").ap()
    with tile.TileContext(nc) as tc:
        if hwloop:
            with ExitStack() as ctx:
                _emit(ctx, tc, (emb8, nn8, mT8, mab2, fct2d, wkv8, wstep8,
                                wout8, wlkT8, seedw, selw, outp), bl,
                      loop_reps=reps)
        else:
            for _ in range(reps):
                with ExitStack() as ctx:
                    _emit(ctx, tc, (emb8, nn8, mT8, mab2, fct2d, wkv8,
                                    wstep8, wout8, wlkT8, seedw, selw,
                                    outp), bl)
    nc.compile()
    return nc


_cache = {}


def _program(bl, reps=1, hwloop=False):
    key = (bl, reps, hwloop)
    if key not in _cache:
        _cache[key] = _build(bl, reps, hwloop)
    return _cache[key]


def _f8(a):
    return a.astype(mybir.dt.np(F8))


def _prep(embedding, current_nodes, used_capacity, used_battery, current_time,
          mask, W_context):
    b = embedding.shape[0]
    # emb8[b,p,c,n] = emb[b, n, c*128+p]
    embT = np.ascontiguousarray(embedding.transpose(0, 2, 1))  # [B, D, N]
    emb8 = _f8(embT.reshape(b, DC, 128, N).transpose(0, 2, 1, 3))
    # nn8[b,p,c,t] = feat[b, t, c*128+p], rows >= D+3 zero
    cur = np.take_along_axis(embedding, current_nodes.astype(np.int64)[:, :, None],
                             axis=1)
    nnf = np.zeros((b, KQ * 128, T), np.float32)
    nnf[:, :D, :] = cur.transpose(0, 2, 1)
    nnf[:, D, :] = 1.0 - used_capacity
    nnf[:, D + 1, :] = 1.0 - used_battery
    nnf[:, D + 2, :] = current_time
    nn8 = _f8(nnf.reshape(b, KQ, 128, T).transpose(0, 2, 1, 3))
    # mT8[b, k or 64+k, i, c*T+t] = MSEED * mask[b, t, c*128 + k + 64*i]
    maT = mask.transpose(0, 2, 1).astype(np.float32) * np.float32(MSEED)
    mT = maT.reshape(b, DC, 2, 64, T).transpose(0, 3, 2, 1, 4).reshape(b, 64, 2, 4 * T)
    mT8 = _f8(np.concatenate([mT, mT], axis=1))  # duplicate rows for PE pairing
    # mab2[pair, t, j, n] = mask[2*pair+j, t, n]  (0/1; scaled -1e8 on device)
    mab2 = np.ascontiguousarray(
        mask.reshape(b // 2, 2, T, N).transpose(0, 2, 1, 3)
    ).astype(ml_dtypes.bfloat16)
    # host fixed context (unscaled; device applies the 1/8 with the query):
    # fct2d[pair, j, m, i] = fc[2*pair + j, m*128 + i]
    fc = (embedding.mean(axis=1) @ W_context).astype(np.float32)  # [B, H]
    fct2d = fc.reshape(b // 2, 2, DC, 128).astype(ml_dtypes.bfloat16)
    return emb8, nn8, mT8, mab2, fct2d


def _prep_weights(W_kvlogit, W_step, W_out):
    wkv8 = _f8(W_kvlogit.reshape(DC, 128, 3 * H).transpose(1, 0, 2))
    ws = np.zeros((KQ * 128, H), np.float32)
    ws[: D + 3] = W_step
    wstep8 = _f8(ws.reshape(KQ, 128, H).transpose(1, 0, 2))
    wout8 = _f8(W_out.reshape(DC, 128, H).transpose(1, 0, 2))
    # wlkT8[p,c,d] = W_lk[d, c*128+p]
    wlk = W_kvlogit[:, 2 * H :]  # [D, H]
    wlkT8 = _f8(np.ascontiguousarray(wlk.T).reshape(DC, 128, D).transpose(1, 0, 2))
    z = np.zeros((64, 2, 128), np.float32)
    for i in range(2):
        z[np.arange(64), i, np.arange(64) + 64 * i] = 1.0
    seedw = _f8(np.concatenate([z, z], axis=0))
    selw = np.zeros((2, 2 * T), np.float32)
    selw[0, 0:T] = 1.0
    selw[1, T:] = 1.0
    selw = selw.astype(ml_dtypes.bfloat16)
    return wkv8, wstep8, wout8, wlkT8, seedw, selw


def prep_in_maps(inputs):
    """Full harness inputs -> per-core input maps for the device program."""
    embedding = np.asarray(inputs["embedding"], np.float32)
    mask = np.asarray(inputs["mask"], bool)
    emb8, nn8, mT8, mab2, fct2d = _prep(
        embedding, np.asarray(inputs["current_nodes"]),
        np.asarray(inputs["used_capacity"], np.float32),
        np.asarray(inputs["used_battery"], np.float32),
        np.asarray(inputs["current_time"], np.float32), mask,
        np.asarray(inputs["W_context"], np.float32))
    wkv8, wstep8, wout8, wlkT8, seedw, selw = _prep_weights(
        np.asarray(inputs["W_kvlogit"], np.float32),
        np.asarray(inputs["W_step"], np.float32),
        np.asarray(inputs["W_out"], np.float32))
    in_maps = []
    for c in range(NCORES):
        s = slice(c * BL, (c + 1) * BL)
        sp = slice(c * BL // 2, (c + 1) * BL // 2)
        in_maps.append({"emb8": emb8[s], "nn8": nn8[s], "mT8": mT8[s],
                        "mab2": mab2[sp], "fct2d": fct2d[sp],
                        "wkv8": wkv8, "wstep8": wstep8, "wout8": wout8,
                        "wlkT8": wlkT8, "seedw": seedw, "selw": selw})
    return in_maps


def kernel(embedding, current_nodes, used_capacity, used_battery, current_time,
           mask, W_context, W_kvlogit, W_step, W_out):
    global LAST_EXEC_TIME_NS
    in_maps = prep_in_maps(dict(
        embedding=embedding, current_nodes=current_nodes,
        used_capacity=used_capacity, used_battery=used_battery,
        current_time=current_time, mask=mask, W_context=W_context,
        W_kvlogit=W_kvlogit, W_step=W_step, W_out=W_out))
    nc = _program(BL)
    res = run_bass_kernel_spmd(nc, in_maps, list(range(NCORES)))
    LAST_EXEC_TIME_NS = res.exec_time_ns
    return np.concatenate([res.results[c]["logp"] for c in range(NCORES)], axis=0)
